# revision 13
# baseline (speedup 1.0000x reference)
"""Trainium2 Bass kernel for nn_MergeDecoder (GNN message passing).

Distribution (8 NeuronCores):
  - stage 1 (dominant, memory-bound): child_W [64,2048,2048] fp32 is sharded
    expert-parallel over the children axis -> 8 children / core. Each core
    streams its 128 MiB of child_W through the PE as the *moving* matmul
    operand (float32r, 1 cycle/row) against the stationary parent vector.
  - AllGather of the resulting node features x [64,2048] (64 KiB per rank).
  - GIN MLPs tensor-parallel over the 2048 hidden dim: W1a/W2a column-sharded
    [2048,256], W1b/W2b row-sharded [256,2048], partial outputs AllReduced.
  - BatchNorm (global batch stats over the 64 nodes) computed redundantly on
    every core in transposed layout (features on partitions, nodes on the
    free axis) via bn_stats/bn_aggr.

Everything after stage 1 stays in T-layout [feature, node] so weight matrices
feed the PE in their natural [in, out] layout as lhsT and biases / BN
parameters are per-partition scalars.
"""

import numpy as np

import concourse.bacc as bacc
import concourse.mybir as mybir
import concourse.tile as tile
from concourse.bass_utils import run_bass_kernel_spmd

NCORES = 8
C = 64          # nodes (children)
F = 2048        # feature size
H = 2048        # hidden size
CPC = C // NCORES   # children per core
HS = H // NCORES    # hidden shard (tensor parallel)
KC = F // 128       # 128-row chunks of the feature dim
MS = HS // 128      # 128-row chunks of the hidden shard
BN_EPS = 1e-5

F32 = mybir.dt.float32
F32R = mybir.dt.float32r
AX = mybir.AxisListType
ALU = mybir.AluOpType
ACT = mybir.ActivationFunctionType


def _gin_layer(nc, tc, pools, hT, wa_sb, ba_sb, wb_sb, wname):
    """T-layout GIN MLP with tensor-parallel hidden shard.

    hT: SBUF [128, KC*64]  (h transposed, chunk-major)
    returns gout_sb [128, KC*64] = partial (h @ Wa |> relu+ba) @ Wb (this
    rank's contribution, pre-AllReduce, no output bias).
    """
    acts, pmm = pools
    aT = acts.tile([128, MS * C], F32, name=f"aT_{wname}")
    for m in range(MS):
        pa = pmm.tile([128, C], F32, name="pa", tag="pa")
        for k in range(KC):
            nc.tensor.matmul(
                pa[:, :],
                lhsT=wa_sb[:, k * HS + m * 128 : k * HS + (m + 1) * 128],
                rhs=hT[:, k * C : (k + 1) * C],
                start=(k == 0),
                stop=(k == KC - 1),
            )
        nc.scalar.activation(
            aT[:, m * C : (m + 1) * C], pa[:, :], ACT.Relu,
            bias=ba_sb[:, m : m + 1], scale=1.0,
        )
    gout = acts.tile([128, KC * C], F32, name=f"gout_{wname}")
    for m in range(KC):
        po = pmm.tile([128, C], F32, name="po", tag="po")
        for j in range(MS):
            nc.tensor.matmul(
                po[:, :],
                lhsT=wb_sb[:, j * F + m * 128 : j * F + (m + 1) * 128],
                rhs=aT[:, j * C : (j + 1) * C],
                start=(j == 0),
                stop=(j == MS - 1),
            )
        nc.scalar.copy(gout[:, m * C : (m + 1) * C], po[:, :])
    return gout


def _bias_relu_bn(nc, tc, pools, ssum, b_sb, g_sb, be_sb, eps, name):
    """t = relu(ssum + b); y = BN(t) with stats over the node (free) axis."""
    acts, pmm = pools
    t = acts.tile([128, KC * C], F32, name=f"t_{name}")
    for k in range(KC):
        nc.scalar.activation(
            t[:, k * C : (k + 1) * C], ssum[:, k * C : (k + 1) * C], ACT.Relu,
            bias=b_sb[:, k : k + 1], scale=1.0,
        )
    mv = acts.tile([128, 2 * KC], F32, name=f"mv_{name}")
    for k in range(KC):
        st6 = acts.tile([128, 6], F32, name="st6", tag="st6", bufs=2)
        nc.vector.bn_stats(st6[:, :], t[:, k * C : (k + 1) * C])
        nc.vector.bn_aggr(mv[:, 2 * k : 2 * k + 2], st6[:, :])
    mean_all = mv[:, 0 : 2 * KC : 2]
    var_all = mv[:, 1 : 2 * KC : 2]
    std = acts.tile([128, KC], F32, name=f"std_{name}")
    nc.scalar.activation(std[:, :], var_all, ACT.Sqrt, bias=eps, scale=1.0)
    inv = acts.tile([128, KC], F32, name=f"inv_{name}")
    nc.vector.reciprocal(inv[:, :], std[:, :])
    scale = acts.tile([128, KC], F32, name=f"scale_{name}")
    nc.vector.scalar_tensor_tensor(
        scale[:, :], in0=inv[:, :], scalar=1.0, in1=g_sb[:, :],
        op0=ALU.mult, op1=ALU.mult,
    )
    shift = acts.tile([128, KC], F32, name=f"shift_{name}")
    nc.vector.scalar_tensor_tensor(
        shift[:, :], in0=mean_all, scalar=-1.0, in1=scale[:, :],
        op0=ALU.mult, op1=ALU.mult,
    )
    nc.vector.scalar_tensor_tensor(
        shift[:, :], in0=shift[:, :], scalar=1.0, in1=be_sb[:, :],
        op0=ALU.mult, op1=ALU.add,
    )
    y = acts.tile([128, KC * C], F32, name=f"y_{name}")
    for k in range(KC):
        nc.scalar.activation(
            y[:, k * C : (k + 1) * C], t[:, k * C : (k + 1) * C], ACT.Identity,
            bias=shift[:, k : k + 1], scale=scale[:, k : k + 1],
        )
    return y


def _agg_h(nc, tc, pools, xT, name):
    """hT = xT + (sum over nodes of xT) broadcast; node 0 receives nothing."""
    acts, pmm = pools
    agg = acts.tile([128, KC], F32, name=f"agg_{name}")
    hT = acts.tile([128, KC * C], F32, name=f"hT_{name}")
    for k in range(KC):
        nc.vector.tensor_reduce(
            agg[:, k : k + 1], xT[:, k * C : (k + 1) * C], axis=AX.X, op=ALU.add,
        )
        nc.scalar.copy(hT[:, k * C : k * C + 1], xT[:, k * C : k * C + 1])
        nc.vector.tensor_scalar_add(
            hT[:, k * C + 1 : (k + 1) * C],
            in0=xT[:, k * C + 1 : (k + 1) * C],
            scalar1=agg[:, k : k + 1],
        )
    return hT


def build_nc():
    nc = bacc.Bacc("TRN2", target_bir_lowering=False, debug=False,
                   num_devices=NCORES)
    pT_d = nc.dram_tensor("pT", [128, KC], F32R, kind="ExternalInput")
    cw_d = nc.dram_tensor("cw", [CPC, F, F], F32R, kind="ExternalInput")
    cb_d = nc.dram_tensor("cb", [CPC, F], F32, kind="ExternalInput")
    w1a_d = nc.dram_tensor("w1a", [F, HS], F32, kind="ExternalInput")
    b1a_d = nc.dram_tensor("b1a", [128, MS], F32, kind="ExternalInput")
    w1b_d = nc.dram_tensor("w1b", [HS, F], F32, kind="ExternalInput")
    b1b_d = nc.dram_tensor("b1b", [128, KC], F32, kind="ExternalInput")
    g1_d = nc.dram_tensor("g1", [128, KC], F32, kind="ExternalInput")
    be1_d = nc.dram_tensor("be1", [128, KC], F32, kind="ExternalInput")
    w2a_d = nc.dram_tensor("w2a", [F, HS], F32, kind="ExternalInput")
    b2a_d = nc.dram_tensor("b2a", [128, MS], F32, kind="ExternalInput")
    w2b_d = nc.dram_tensor("w2b", [HS, F], F32, kind="ExternalInput")
    b2b_d = nc.dram_tensor("b2b", [128, KC], F32, kind="ExternalInput")
    g2_d = nc.dram_tensor("g2", [128, KC], F32, kind="ExternalInput")
    be2_d = nc.dram_tensor("be2", [128, KC], F32, kind="ExternalInput")
    id_d = nc.dram_tensor("ident", [C, C], F32, kind="ExternalInput")
    out_d = nc.dram_tensor("outT", [128, KC, C], F32, kind="ExternalOutput")

    with tile.TileContext(nc) as tc:
        with (
            tc.tile_pool(name="consts", bufs=1) as consts,
            tc.tile_pool(name="ginw", bufs=1) as ginw,
            tc.tile_pool(name="acts", bufs=1) as acts,
            tc.tile_pool(name="dram", bufs=1, space="DRAM") as dram,
        ):
            # ---- constants + TP weight shards (prefetched during stage 1)
            pT = consts.tile([128, KC], F32R, name="pT_sb")
            nc.sync.dma_start(pT[:, :], pT_d[:, :])

            ident = consts.tile([C, C], F32, name="ident_sb")
            nc.sync.dma_start(ident[:, :], id_d[:, :])
            eps_t = consts.tile([128, 1], F32, name="eps_sb")
            nc.gpsimd.memset(eps_t[:, :], BN_EPS)
            eps = eps_t[:, :]

            small = {}
            for nm, d in [("b1a", b1a_d), ("b1b", b1b_d), ("g1", g1_d),
                          ("be1", be1_d), ("b2a", b2a_d), ("b2b", b2b_d),
                          ("g2", g2_d), ("be2", be2_d)]:
                s = consts.tile(list(d.shape), F32, name=f"{nm}_sb")
                nc.sync.dma_start(s[:, :], d[:, :])
                small[nm] = s

            w1a = ginw.tile([128, KC * HS], F32, name="w1a_sb")
            w2a = ginw.tile([128, KC * HS], F32, name="w2a_sb")
            for k in range(KC):
                nc.sync.dma_start(w1a[:, k * HS : (k + 1) * HS],
                                  w1a_d[k * 128 : (k + 1) * 128, :])
                nc.sync.dma_start(w2a[:, k * HS : (k + 1) * HS],
                                  w2a_d[k * 128 : (k + 1) * 128, :])
            w1b = ginw.tile([128, MS * F], F32, name="w1b_sb")
            w2b = ginw.tile([128, MS * F], F32, name="w2b_sb")
            for j in range(MS):
                nc.sync.dma_start(w1b[:, j * F : (j + 1) * F],
                                  w1b_d[j * 128 : (j + 1) * 128, :])
                nc.sync.dma_start(w2b[:, j * F : (j + 1) * F],
                                  w2b_d[j * 128 : (j + 1) * 128, :])

            # ---- stage 1: x[c] = relu(parent @ child_W[c] + child_b[c])
            # each finished row is DMAed straight into the AllGather input
            ag_in = dram.tile([CPC, F], F32, name="ag_in")
            ag_out = dram.tile([C, F], F32, name="ag_out", addr_space="Shared")
            with (
                tc.tile_pool(name="wstream", bufs=4) as wstream,
                tc.tile_pool(name="prow", bufs=2, space="PSUM") as prow,
                tc.tile_pool(name="rowtmp", bufs=2) as rowtmp,
                tc.tile_pool(name="cbrow", bufs=2) as cbpool,
                tc.tile_pool(name="xrow", bufs=2) as xpool,
            ):
                for j in range(CPC):
                    pr = prow.tile([1, F], F32, name="pr", tag="pr")
                    for k in range(KC):
                        wt = wstream.tile([128, F], F32R, name="wt", tag="wt")
                        nc.sync.dma_start(
                            wt[:, :], cw_d[j, k * 128 : (k + 1) * 128, :])
                        for n in range(F // 512):
                            nc.tensor.matmul(
                                pr[0:1, n * 512 : (n + 1) * 512],
                                lhsT=pT[:, k : k + 1],
                                rhs=wt[:, n * 512 : (n + 1) * 512],
                                start=(k == 0),
                                stop=(k == KC - 1),
                            )
                    cbrow = cbpool.tile([1, F], F32, name="cbrow", tag="cbrow")
                    nc.sync.dma_start(cbrow[:, :], cb_d[j : j + 1, :])
                    tmp = rowtmp.tile([1, F], F32, name="tmp", tag="tmp")
                    nc.vector.scalar_tensor_tensor(
                        tmp[:, :], in0=pr[0:1, :], scalar=1.0,
                        in1=cbrow[:, :], op0=ALU.mult, op1=ALU.add,
                    )
                    xrow = xpool.tile([1, F], F32, name="xrow", tag="xrow")
                    nc.scalar.activation(xrow[:, :], tmp[:, :], ACT.Relu)
                    nc.sync.dma_start(ag_in[j : j + 1, :], xrow[:, :])

            # ---- AllGather x across the 8 cores
            nc.gpsimd.collective_compute(
                "AllGather", ALU.bypass,
                replica_groups=[list(range(NCORES))],
                ins=[ag_in[:, :].opt()], outs=[ag_out[:, :].opt()],
            )
            xfull = acts.tile([C, F], F32, name="xfull_sb")
            nc.sync.dma_start(xfull[:, :], ag_out[:, :])

            with tc.tile_pool(name="pmm", bufs=2, space="PSUM") as pmm:
                pools = (acts, pmm)
                # transpose to T-layout xT [feature, node]
                xT = acts.tile([128, KC * C], F32, name="xT_sb")
                for k in range(KC):
                    ptx = pmm.tile([128, C], F32, name="ptx", tag="ptx")
                    nc.tensor.transpose(
                        ptx[:, :], xfull[:, k * 128 : (k + 1) * 128],
                        ident[:, :])
                    nc.scalar.copy(xT[:, k * C : (k + 1) * C], ptx[:, :])

                # ---- GIN layer 1
                h1 = _agg_h(nc, tc, pools, xT, "l1")
                g1out = _gin_layer(nc, tc, pools, h1, w1a, small["b1a"],
                                   w1b, "l1")
                ar1_in = dram.tile([128, KC * C], F32, name="ar1_in")
                ar1_out = dram.tile([128, KC * C], F32, name="ar1_out",
                                    addr_space="Shared")
                nc.sync.dma_start(ar1_in[:, :], g1out[:, :])
                nc.gpsimd.collective_compute(
                    "AllReduce", ALU.add,
                    replica_groups=[list(range(NCORES))],
                    ins=[ar1_in[:, :].opt()], outs=[ar1_out[:, :].opt()],
                )
                s1 = acts.tile([128, KC * C], F32, name="s1_sb")
                nc.sync.dma_start(s1[:, :], ar1_out[:, :])
                y1 = _bias_relu_bn(nc, tc, pools, s1, small["b1b"],
                                   small["g1"], small["be1"], eps, "l1")

                # ---- GIN layer 2
                h2 = _agg_h(nc, tc, pools, y1, "l2")
                g2out = _gin_layer(nc, tc, pools, h2, w2a, small["b2a"],
                                   w2b, "l2")
                ar2_in = dram.tile([128, KC * C], F32, name="ar2_in")
                ar2_out = dram.tile([128, KC * C], F32, name="ar2_out",
                                    addr_space="Shared")
                nc.sync.dma_start(ar2_in[:, :], g2out[:, :])
                nc.gpsimd.collective_compute(
                    "AllReduce", ALU.add,
                    replica_groups=[list(range(NCORES))],
                    ins=[ar2_in[:, :].opt()], outs=[ar2_out[:, :].opt()],
                )
                s2 = acts.tile([128, KC * C], F32, name="s2_sb")
                nc.sync.dma_start(s2[:, :], ar2_out[:, :])
                y2 = _bias_relu_bn(nc, tc, pools, s2, small["b2b"],
                                   small["g2"], small["be2"], eps, "l2")

                nc.sync.dma_start(out_d[:, :, :], y2[:, :])
    nc.finalize()
    return nc


def _colmajor(v, cols):
    """[cols*128] vector -> [128, cols] with column k = v[k*128:(k+1)*128]."""
    return np.ascontiguousarray(np.asarray(v, np.float32).reshape(cols, 128).T)


def prepare_in_maps(inputs):
    f32 = np.float32
    parent = np.asarray(inputs["parent_feature"], f32).reshape(-1)
    child_W = np.asarray(inputs["child_W"], f32)
    child_b = np.asarray(inputs["child_b"], f32)
    W1a = np.asarray(inputs["W1a"], f32)
    W1b = np.asarray(inputs["W1b"], f32)
    W2a = np.asarray(inputs["W2a"], f32)
    W2b = np.asarray(inputs["W2b"], f32)

    pT = _colmajor(parent, KC)
    ident = np.eye(C, dtype=f32)
    b1bT = _colmajor(inputs["b1b"], KC)
    g1T = _colmajor(inputs["g1"], KC)
    be1T = _colmajor(inputs["beta1"], KC)
    b2bT = _colmajor(inputs["b2b"], KC)
    g2T = _colmajor(inputs["g2"], KC)
    be2T = _colmajor(inputs["beta2"], KC)
    b1a = np.asarray(inputs["b1a"], f32)
    b2a = np.asarray(inputs["b2a"], f32)

    in_maps = []
    for r in range(NCORES):
        sl = slice(r * HS, (r + 1) * HS)
        in_maps.append({
            "pT": pT,
            "cw": np.ascontiguousarray(child_W[r * CPC : (r + 1) * CPC]),
            "cb": np.ascontiguousarray(child_b[r * CPC : (r + 1) * CPC]),
            "w1a": np.ascontiguousarray(W1a[:, sl]),
            "b1a": _colmajor(b1a[sl], MS),
            "w1b": np.ascontiguousarray(W1b[sl, :]),
            "b1b": b1bT, "g1": g1T, "be1": be1T,
            "w2a": np.ascontiguousarray(W2a[:, sl]),
            "b2a": _colmajor(b2a[sl], MS),
            "w2b": np.ascontiguousarray(W2b[sl, :]),
            "b2b": b2bT, "g2": g2T, "be2": be2T,
            "ident": ident,
        })
    return in_maps


_NC_CACHE = {}


def get_nc():
    if "nc" not in _NC_CACHE:
        _NC_CACHE["nc"] = build_nc()
    return _NC_CACHE["nc"]


def unpack_out(outT):
    # outT [128, KC, C] with outT[p, k, c] = out[c, k*128 + p]
    return np.ascontiguousarray(
        np.asarray(outT).transpose(1, 0, 2).reshape(F, C).T)


def kernel(**inputs):
    nc = get_nc()
    in_maps = prepare_in_maps(inputs)
    res = run_bass_kernel_spmd(nc, in_maps, core_ids=list(range(NCORES)))
    return unpack_out(res.results[0]["outT"])
